# revision 3
# baseline (speedup 1.0000x reference)
"""Trainium2 Bass kernel for GNN NodeBlock (segment_sum + MLP), 8-core SPMD.

Strategy (node-sharded):
  - Shard the 100k nodes across 8 cores (12500 each). Host buckets edges by
    receiver core and by 128-node group within the core, padding each group's
    edge list to a multiple of 128 (pad idx = -1).
  - Device, per 128-node group: build one-hot(edge -> node-in-group) matrices
    with a DVE is_equal against an iota, then matmul-accumulate
    onehot^T @ edge_hilo into PSUM. Edge features travel as bf16 (hi, lo)
    pairs so the f32 value is reconstructed exactly in the f32 PSUM
    accumulator: agg = sum(hi) + sum(lo).
  - The aggregate is PE-transposed to feature-major [32, nodes] and fed,
    together with host-pre-transposed node features, through the two-layer
    MLP with the weights stationary. The globals term is folded into b1 on
    the host (g @ W1[64:96] is the same for every node).
  - No collectives: cores own disjoint node ranges; host concatenates.
"""

import os

import numpy as np
import ml_dtypes

import concourse.bacc as bacc
import concourse.bass as bass
import concourse.mybir as mybir
import concourse.tile as tile
from concourse.bass_utils import run_bass_kernel_spmd
from concourse.masks import make_identity

BF16 = ml_dtypes.bfloat16

N_NODES = 100000
N_CORES = 8
NPC = N_NODES // N_CORES  # 12500 nodes per core
P = 128
G = -(-NPC // P)  # 98 groups of 128 nodes per core
D = 32

_prog_cache = {}


def _host_prep(node_attr, edge_index, edge_attr, global_attr, W1, b1, W2, b2):
    E = edge_attr.shape[0]
    r = np.ascontiguousarray(edge_index[1]).astype(np.int64)
    core = r // NPC
    local = r - core * NPC
    lg = local // P  # group within core
    w = (local - lg * P).astype(np.int32)  # node within group
    key = (core * G + lg).astype(np.int64)  # 0 .. 8*98-1

    order = np.argsort(key, kind="stable")
    key_s = key[order]
    w_s = w[order]

    counts = np.bincount(key, minlength=N_CORES * G)
    m = int(-(-counts.max() // P))  # 128-edge tiles per group
    cap = m * P

    starts = np.zeros(N_CORES * G, dtype=np.int64)
    np.cumsum(counts[:-1], out=starts[1:])
    slot = np.arange(E, dtype=np.int64) - starts[key_s]

    ea = np.ascontiguousarray(edge_attr, dtype=np.float32)
    hi = ea.astype(BF16)
    lo = (ea - hi.astype(np.float32)).astype(BF16)

    buf = np.zeros((N_CORES * G, cap, 2 * D), dtype=BF16)
    buf[key_s, slot, :D] = hi[order]
    buf[key_s, slot, D:] = lo[order]
    # (bucket, t, p, 64) -> (core, G, p, m*64): partition-major per group
    edges_arr = np.ascontiguousarray(
        buf.reshape(N_CORES, G, m, P, 2 * D).transpose(0, 1, 3, 2, 4)
    ).reshape(N_CORES, G, P, m * 2 * D)

    widx = np.full((N_CORES * G, cap), -1.0, dtype=np.float32)
    widx[key_s, slot] = w_s.astype(np.float32)
    # (bucket, t, p) -> (core, p, G*m)
    idx_arr = np.ascontiguousarray(
        widx.reshape(N_CORES, G, m, P).transpose(0, 3, 1, 2)
    ).reshape(N_CORES, P, G * m).astype(BF16)

    nodeT = np.ascontiguousarray(
        node_attr.astype(np.float32).reshape(N_CORES, NPC, D).transpose(0, 2, 1)
    )

    g0 = global_attr.astype(np.float32).reshape(1, D)
    W1 = W1.astype(np.float32)
    b1p = (b1.astype(np.float32) + (g0 @ W1[2 * D :]).reshape(-1)).reshape(D, 1)
    W1a = np.ascontiguousarray(W1[:D])  # [32, 32]
    W1b = np.ascontiguousarray(W1[D : 2 * D])  # [32, 32]
    W2 = np.ascontiguousarray(W2.astype(np.float32))  # [32, 32]
    b2 = b2.astype(np.float32).reshape(D, 1)

    in_maps = []
    for c in range(N_CORES):
        in_maps.append(
            {
                "edges": edges_arr[c],
                "idx": idx_arr[c],
                "nodeT": nodeT[c],
                "w1a": W1a,
                "w1b": W1b,
                "w2": W2,
                "b1p": b1p,
                "b2": b2,
            }
        )
    return in_maps, m


def _build_program(m):
    if m in _prog_cache:
        return _prog_cache[m]

    f32 = mybir.dt.float32
    bf16 = mybir.dt.bfloat16
    nc = bacc.Bacc(
        "TRN2", target_bir_lowering=False, debug=False, num_devices=N_CORES
    )

    edges_d = nc.dram_tensor("edges", [G, P, m * 2 * D], bf16, kind="ExternalInput")
    idx_d = nc.dram_tensor("idx", [P, G * m], bf16, kind="ExternalInput")
    nodeT_d = nc.dram_tensor("nodeT", [D, NPC], f32, kind="ExternalInput")
    w1a_d = nc.dram_tensor("w1a", [D, D], f32, kind="ExternalInput")
    w1b_d = nc.dram_tensor("w1b", [D, D], f32, kind="ExternalInput")
    w2_d = nc.dram_tensor("w2", [D, D], f32, kind="ExternalInput")
    b1p_d = nc.dram_tensor("b1p", [D, 1], f32, kind="ExternalInput")
    b2_d = nc.dram_tensor("b2", [D, 1], f32, kind="ExternalInput")
    outT_d = nc.dram_tensor("outT", [D, NPC], f32, kind="ExternalOutput")

    with tile.TileContext(nc) as tc:
        with (
            tc.tile_pool(name="const", bufs=1) as cpool,
            tc.tile_pool(name="edges", bufs=3) as epool,
            tc.tile_pool(name="oh", bufs=3) as opool,
            tc.tile_pool(name="mlp", bufs=2) as mpool,
            tc.tile_pool(name="psum", bufs=2, space="PSUM") as pspool,
        ):
            # constants
            iota32 = cpool.tile([P, P], mybir.dt.int32)
            nc.gpsimd.iota(iota32[:], pattern=[[1, P]], base=0, channel_multiplier=0)
            iotab = cpool.tile([P, P], bf16)
            nc.vector.tensor_copy(out=iotab[:], in_=iota32[:])
            identity = cpool.tile([P, P], f32)
            make_identity(nc, identity[:])

            idx_all = cpool.tile([P, G * m], bf16)
            nc.sync.dma_start(out=idx_all[:], in_=idx_d.ap())
            nodeT_sb = cpool.tile([D, NPC], f32)
            nc.sync.dma_start(out=nodeT_sb[:], in_=nodeT_d.ap())
            w1a_sb = cpool.tile([D, D], f32)
            nc.sync.dma_start(out=w1a_sb[:], in_=w1a_d.ap())
            w1b_sb = cpool.tile([D, D], f32)
            nc.sync.dma_start(out=w1b_sb[:], in_=w1b_d.ap())
            w2_sb = cpool.tile([D, D], f32)
            nc.sync.dma_start(out=w2_sb[:], in_=w2_d.ap())
            b1p_sb = cpool.tile([D, 1], f32)
            nc.sync.dma_start(out=b1p_sb[:], in_=b1p_d.ap())
            b2_sb = cpool.tile([D, 1], f32)
            nc.sync.dma_start(out=b2_sb[:], in_=b2_d.ap())

            aggT = cpool.tile([D, G * P], f32)  # [32, 12544]

            for g in range(G):
                edges_t = epool.tile([P, m * 2 * D], bf16)
                nc.sync.dma_start(out=edges_t[:], in_=edges_d.ap()[g])
                oh = opool.tile([P, m, P], bf16)
                nc.vector.tensor_tensor(
                    out=oh[:],
                    in0=idx_all[:, g * m : (g + 1) * m].to_broadcast([P, m, P]),
                    in1=iotab[:][:, None, :].to_broadcast([P, m, P]),
                    op=mybir.AluOpType.is_equal,
                )
                ps = pspool.tile([P, 2 * D], f32)
                for t in range(m):
                    nc.tensor.matmul(
                        out=ps[:],
                        lhsT=oh[:, t, :],
                        rhs=edges_t[:, t * 2 * D : (t + 1) * 2 * D],
                        start=(t == 0),
                        stop=(t == m - 1),
                    )
                ps2 = opool.tile([P, 2 * D], f32, tag="ps2")
                nc.scalar.activation(
                    out=ps2[:], in_=ps[:], func=mybir.ActivationFunctionType.Copy
                )
                agg_g = opool.tile([P, D], f32, tag="agg_g")
                nc.vector.tensor_tensor(
                    out=agg_g[:],
                    in0=ps2[:, :D],
                    in1=ps2[:, D:],
                    op=mybir.AluOpType.add,
                )
                pt = pspool.tile([D, P], f32)
                nc.tensor.transpose(out=pt[:], in_=agg_g[:], identity=identity[:])
                nc.scalar.activation(
                    out=aggT[:, g * P : (g + 1) * P],
                    in_=pt[:],
                    func=mybir.ActivationFunctionType.Copy,
                )

            # MLP over node chunks
            CH = 512
            n_done = 0
            while n_done < NPC:
                n = min(CH, NPC - n_done)
                cols = slice(n_done, n_done + n)
                ph = pspool.tile([D, CH], f32, tag="ph")
                nc.tensor.matmul(
                    out=ph[:, :n],
                    lhsT=w1a_sb[:],
                    rhs=nodeT_sb[:, cols],
                    start=True,
                    stop=False,
                )
                nc.tensor.matmul(
                    out=ph[:, :n],
                    lhsT=w1b_sb[:],
                    rhs=aggT[:, cols],
                    start=False,
                    stop=True,
                )
                hT = mpool.tile([D, CH], f32, tag="hT")
                nc.scalar.activation(
                    out=hT[:, :n],
                    in_=ph[:, :n],
                    func=mybir.ActivationFunctionType.Relu,
                    bias=b1p_sb[:],
                    scale=1.0,
                )
                po = pspool.tile([D, CH], f32, tag="po")
                nc.tensor.matmul(
                    out=po[:, :n],
                    lhsT=w2_sb[:],
                    rhs=hT[:, :n],
                    start=True,
                    stop=True,
                )
                ot = mpool.tile([D, CH], f32, tag="ot")
                nc.vector.tensor_tensor(
                    out=ot[:, :n],
                    in0=po[:, :n],
                    in1=b2_sb[:].to_broadcast([D, n]),
                    op=mybir.AluOpType.add,
                )
                nc.sync.dma_start(out=outT_d.ap()[:, cols], in_=ot[:, :n])
                n_done += n

    nc.finalize()
    _prog_cache[m] = nc
    return nc


def kernel(**inputs):
    in_maps, m = _host_prep(**inputs)
    nc = _build_program(m)
    trace = bool(os.environ.get("KERNEL_TRACE"))
    res = run_bass_kernel_spmd(nc, in_maps, list(range(N_CORES)), trace=trace)
    if trace:
        print(f"HW exec time: {res.exec_time_ns} ns")
        print(f"mean exec time: {res.mean_exec_time_ns} ns")
    out = np.empty((N_NODES, D), dtype=np.float32)
    for c in range(N_CORES):
        out[c * NPC : (c + 1) * NPC] = res.results[c]["outT"].T
    return out


# revision 5
# speedup vs baseline: 1.0499x; 1.0499x over previous
"""Trainium2 Bass kernel for GNN NodeBlock (segment_sum + MLP), 8-core SPMD.

Strategy (node-sharded):
  - Shard the 100k nodes across 8 cores (12500 each). Host buckets edges by
    receiver core and by 128-node group within the core, padding each group's
    edge list to a multiple of 128 (pad idx = -1).
  - Device, per 128-node group: build one-hot(edge -> node-in-group) matrices
    with an is_equal against an iota (split between DVE and GPSIMD), then
    matmul-accumulate onehot^T @ edge_hilo into PSUM. Edge features travel as
    bf16 (hi, lo) pairs so the f32 value is reconstructed exactly in the f32
    PSUM accumulator: agg = sum(hi) + sum(lo).
  - The aggregate is PE-transposed to feature-major and stored as bf16
    (hi, lo) halves; the MLP runs with K-stacked bf16 matmuls (exact to
    ~2^-16) with the weights' own hi/lo splits baked in on the host. The
    globals term is folded into b1 on the host.
  - No collectives: cores own disjoint node ranges; host concatenates.
"""

import os

import numpy as np
import ml_dtypes

import concourse.bacc as bacc
import concourse.bass as bass
import concourse.mybir as mybir
import concourse.tile as tile
from concourse.bass_utils import run_bass_kernel_spmd
from concourse.masks import make_identity

BF16 = ml_dtypes.bfloat16

N_NODES = 100000
N_CORES = 8
NPC = N_NODES // N_CORES  # 12500 nodes per core
P = 128
GR = -(-NPC // P)  # 98 real groups of 128 nodes per core
BG = 4  # groups per DMA batch
NB = -(-GR // BG)  # 25 batches
G = NB * BG  # 100 groups incl. dummy padding groups
D = 32

_prog_cache = {}


def _split_hi_lo(x):
    hi = x.astype(BF16)
    lo = (x - hi.astype(np.float32)).astype(BF16)
    return hi, lo


def _host_prep(node_attr, edge_index, edge_attr, global_attr, W1, b1, W2, b2):
    E = edge_attr.shape[0]
    r = np.ascontiguousarray(edge_index[1]).astype(np.int64)
    core = r // NPC
    local = r - core * NPC
    lg = local // P  # group within core (0..97)
    w = (local - lg * P).astype(np.int32)  # node within group
    key = (core * G + lg).astype(np.int64)

    order = np.argsort(key, kind="stable")
    key_s = key[order]
    w_s = w[order]

    counts = np.bincount(key, minlength=N_CORES * G)
    m = int(-(-counts.max() // P))  # 128-edge tiles per group
    cap = m * P

    starts = np.zeros(N_CORES * G, dtype=np.int64)
    np.cumsum(counts[:-1], out=starts[1:])
    slot = np.arange(E, dtype=np.int64) - starts[key_s]

    ea = np.ascontiguousarray(edge_attr, dtype=np.float32)
    hi, lo = _split_hi_lo(ea)

    buf = np.zeros((N_CORES * G, cap, 2 * D), dtype=BF16)
    buf[key_s, slot, :D] = hi[order]
    buf[key_s, slot, D:] = lo[order]
    # (bucket, t, p, 64) -> (core, batch, p, BG*m*64): partition-major per batch
    edges_arr = np.ascontiguousarray(
        buf.reshape(N_CORES, NB, BG, m, P, 2 * D).transpose(0, 1, 4, 2, 3, 5)
    ).reshape(N_CORES, NB, P, BG * m * 2 * D)

    widx = np.full((N_CORES * G, cap), -1.0, dtype=np.float32)
    widx[key_s, slot] = w_s.astype(np.float32)
    # (bucket, t, p) -> (core, p, G*m)
    idx_arr = np.ascontiguousarray(
        widx.reshape(N_CORES, G, m, P).transpose(0, 3, 1, 2)
    ).reshape(N_CORES, P, G * m).astype(BF16)

    nodeT = np.ascontiguousarray(
        node_attr.astype(np.float32).reshape(N_CORES, NPC, D).transpose(0, 2, 1)
    )
    nhi, nlo = _split_hi_lo(nodeT)
    # K-stack [hi; hi; lo] so one K=96 matmul computes n@Whi + n@Wlo exactly
    node_stack = np.concatenate([nhi, nhi, nlo], axis=1)  # (8, 96, NPC)

    g0 = global_attr.astype(np.float32).reshape(1, D)
    W1 = W1.astype(np.float32)
    b1p = (b1.astype(np.float32) + (g0 @ W1[2 * D :]).reshape(-1)).reshape(D, 1)
    w1a_hi, w1a_lo = _split_hi_lo(W1[:D])
    w1b_hi, w1b_lo = _split_hi_lo(W1[D : 2 * D])
    # lhsT stacks matching [hi; hi; lo] rhs: rows [W_hi; W_lo; W_hi]
    w1n = np.ascontiguousarray(np.concatenate([w1a_hi, w1a_lo, w1a_hi], axis=0))
    w1g = np.ascontiguousarray(np.concatenate([w1b_hi, w1b_lo, w1b_hi], axis=0))
    W2 = np.ascontiguousarray(W2.astype(np.float32))
    b2 = b2.astype(np.float32).reshape(D, 1)

    in_maps = []
    for c in range(N_CORES):
        in_maps.append(
            {
                "edges": edges_arr[c],
                "idx": idx_arr[c],
                "nodeS": node_stack[c],
                "w1n": w1n,
                "w1g": w1g,
                "w2": W2,
                "b1p": b1p,
                "b2": b2,
            }
        )
    return in_maps, m


def _build_program(m):
    if m in _prog_cache:
        return _prog_cache[m]

    f32 = mybir.dt.float32
    bf16 = mybir.dt.bfloat16
    nc = bacc.Bacc(
        "TRN2", target_bir_lowering=False, debug=False, num_devices=N_CORES
    )

    edges_d = nc.dram_tensor(
        "edges", [NB, P, BG * m * 2 * D], bf16, kind="ExternalInput"
    )
    idx_d = nc.dram_tensor("idx", [P, G * m], bf16, kind="ExternalInput")
    nodeS_d = nc.dram_tensor("nodeS", [3 * D, NPC], bf16, kind="ExternalInput")
    w1n_d = nc.dram_tensor("w1n", [3 * D, D], bf16, kind="ExternalInput")
    w1g_d = nc.dram_tensor("w1g", [3 * D, D], bf16, kind="ExternalInput")
    w2_d = nc.dram_tensor("w2", [D, D], f32, kind="ExternalInput")
    b1p_d = nc.dram_tensor("b1p", [D, 1], f32, kind="ExternalInput")
    b2_d = nc.dram_tensor("b2", [D, 1], f32, kind="ExternalInput")
    outT_d = nc.dram_tensor("outT", [D, NPC], f32, kind="ExternalOutput")

    with tile.TileContext(nc) as tc:
        with (
            tc.tile_pool(name="const", bufs=1) as cpool,
            tc.tile_pool(name="edges", bufs=3) as epool,
            tc.tile_pool(name="oh", bufs=4) as opool,
            tc.tile_pool(name="mlp", bufs=2) as mpool,
            tc.tile_pool(name="psum", bufs=2, space="PSUM") as pspool,
        ):
            # constants
            iota32 = cpool.tile([P, m, P], mybir.dt.int32)
            nc.gpsimd.iota(
                iota32[:], pattern=[[0, m], [1, P]], base=0, channel_multiplier=0
            )
            iotab = cpool.tile([P, m, P], bf16)
            nc.vector.tensor_copy(out=iotab[:], in_=iota32[:])
            identity = cpool.tile([P, P], f32)
            make_identity(nc, identity[:])

            idx_all = cpool.tile([P, G * m], bf16)
            nc.sync.dma_start(out=idx_all[:], in_=idx_d.ap())
            nodeS_sb = cpool.tile([3 * D, NPC], bf16)
            nc.sync.dma_start(out=nodeS_sb[:], in_=nodeS_d.ap())
            w1n_sb = cpool.tile([3 * D, D], bf16)
            nc.sync.dma_start(out=w1n_sb[:], in_=w1n_d.ap())
            w1g_sb = cpool.tile([3 * D, D], bf16)
            nc.sync.dma_start(out=w1g_sb[:], in_=w1g_d.ap())
            w2_sb = cpool.tile([D, D], f32)
            nc.sync.dma_start(out=w2_sb[:], in_=w2_d.ap())
            b1p_sb = cpool.tile([D, 1], f32)
            nc.sync.dma_start(out=b1p_sb[:], in_=b1p_d.ap())
            b2_sb = cpool.tile([D, 1], f32)
            nc.sync.dma_start(out=b2_sb[:], in_=b2_d.ap())

            # agg stack [hi; hi; lo] built via DMA duplication at the end
            aggS = cpool.tile([3 * D, G * P], bf16)  # [96, 12800]
            aggL = cpool.tile([D, G * P], bf16)  # lo scratch [32, 12800]

            for b in range(NB):
                edges_t = epool.tile([P, BG * m * 2 * D], bf16)
                nc.sync.dma_start(out=edges_t[:], in_=edges_d.ap()[b])
                for j in range(BG):
                    g = b * BG + j
                    if g >= GR:
                        continue  # dummy padding group
                    oh = opool.tile([P, m, P], bf16)
                    nc.vector.tensor_tensor(
                        out=oh[:],
                        in0=idx_all[:, g * m : (g + 1) * m].to_broadcast([P, m, P]),
                        in1=iotab[:],
                        op=mybir.AluOpType.is_equal,
                    )
                    ps = pspool.tile([P, 2 * D], f32)
                    for t in range(m):
                        base = (j * m + t) * 2 * D
                        nc.tensor.matmul(
                            out=ps[:],
                            lhsT=oh[:, t, :],
                            rhs=edges_t[:, base : base + 2 * D],
                            start=(t == 0),
                            stop=(t == m - 1),
                        )
                    ps2 = opool.tile([P, 2 * D], f32, tag="ps2")
                    nc.scalar.activation(
                        out=ps2[:], in_=ps[:], func=mybir.ActivationFunctionType.Copy
                    )
                    agg_g = opool.tile([P, D], f32, tag="agg_g")
                    nc.vector.tensor_tensor(
                        out=agg_g[:],
                        in0=ps2[:, :D],
                        in1=ps2[:, D:],
                        op=mybir.AluOpType.add,
                    )
                    pt = pspool.tile([D, P], f32)
                    nc.tensor.transpose(out=pt[:], in_=agg_g[:], identity=identity[:])
                    cols = slice(g * P, (g + 1) * P)
                    nc.scalar.activation(
                        out=aggS[:D, cols],
                        in_=pt[:],
                        func=mybir.ActivationFunctionType.Copy,
                    )
                    nc.vector.tensor_tensor(
                        out=aggL[:, cols],
                        in0=pt[:],
                        in1=aggS[:D, cols],
                        op=mybir.AluOpType.subtract,
                    )

            # duplicate hi to rows 32:64, lo to rows 64:96 (DMA moves partitions)
            nc.sync.dma_start(out=aggS[D : 2 * D, :], in_=aggS[:D, :])
            nc.sync.dma_start(out=aggS[2 * D :, :], in_=aggL[:])

            # MLP over node chunks
            CH = 512
            n_done = 0
            while n_done < NPC:
                n = min(CH, NPC - n_done)
                cols = slice(n_done, n_done + n)
                ph = pspool.tile([D, CH], f32, tag="ph")
                nc.tensor.matmul(
                    out=ph[:, :n],
                    lhsT=w1n_sb[:],
                    rhs=nodeS_sb[:, cols],
                    start=True,
                    stop=False,
                )
                nc.tensor.matmul(
                    out=ph[:, :n],
                    lhsT=w1g_sb[:],
                    rhs=aggS[:, cols],
                    start=False,
                    stop=True,
                )
                hT = mpool.tile([D, CH], f32, tag="hT")
                nc.scalar.activation(
                    out=hT[:, :n],
                    in_=ph[:, :n],
                    func=mybir.ActivationFunctionType.Relu,
                    bias=b1p_sb[:],
                    scale=1.0,
                )
                po = pspool.tile([D, CH], f32, tag="po")
                nc.tensor.matmul(
                    out=po[:, :n],
                    lhsT=w2_sb[:],
                    rhs=hT[:, :n],
                    start=True,
                    stop=True,
                )
                ot = mpool.tile([D, CH], f32, tag="ot")
                nc.vector.tensor_tensor(
                    out=ot[:, :n],
                    in0=po[:, :n],
                    in1=b2_sb[:].to_broadcast([D, n]),
                    op=mybir.AluOpType.add,
                )
                nc.sync.dma_start(out=outT_d.ap()[:, cols], in_=ot[:, :n])
                n_done += n

    nc.finalize()
    _prog_cache[m] = nc
    return nc


def kernel(**inputs):
    in_maps, m = _host_prep(**inputs)
    nc = _build_program(m)
    trace = bool(os.environ.get("KERNEL_TRACE"))
    res = run_bass_kernel_spmd(nc, in_maps, list(range(N_CORES)), trace=trace)
    if trace:
        print(f"HW exec time: {res.exec_time_ns} ns")
        print(f"mean exec time: {res.mean_exec_time_ns} ns")
    out = np.empty((N_NODES, D), dtype=np.float32)
    for c in range(N_CORES):
        out[c * NPC : (c + 1) * NPC] = res.results[c]["outT"].T
    return out


# revision 6
# speedup vs baseline: 1.8480x; 1.7602x over previous
"""Trainium2 Bass kernel for GNN NodeBlock (segment_sum + MLP), 8-core SPMD.

Strategy (node-sharded, two-path aggregation):
  - Shard the 100k nodes across 8 cores (12500 each). Host sorts edges by
    receiver.
  - Dense path: each node gets 16 padded edge slots (covers ~90% of edges).
    On device the 16 slot-planes are summed by PSUM-accumulated matmuls with
    a constant bf16 identity as the stationary operand (psum += rhs), so the
    PE never reloads weights and no one-hot is needed.
  - Overflow path: edges beyond a node's 16th go through a one-hot matmul
    scatter (is_equal against an iota on DVE), accumulating into the same
    PSUM region.
  - Edge features travel as bf16 (hi, lo) pairs so the f32 value is
    reconstructed exactly in the f32 PSUM accumulator.
  - The per-supergroup aggregate (512 nodes) is hi+lo-combined, PE-transposed
    to feature-major, and stored as bf16 (hi, lo); the MLP runs K-stacked
    bf16 matmuls (exact to ~2^-16) with weight hi/lo splits baked in on the
    host. The globals term is folded into b1 on the host.
  - No collectives: cores own disjoint node ranges; host concatenates.
"""

import os

import numpy as np
import ml_dtypes

import concourse.bacc as bacc
import concourse.bass as bass
import concourse.mybir as mybir
import concourse.tile as tile
from concourse.bass_utils import run_bass_kernel_spmd
from concourse.masks import make_identity

BF16 = ml_dtypes.bfloat16

N_NODES = 100000
N_CORES = 8
NPC = N_NODES // N_CORES  # 12500 nodes per core
P = 128
SG = 512  # nodes per supergroup (4 windows of 128)
NSG = -(-NPC // SG)  # 25 supergroups per core
WPS = SG // P  # 4 windows per supergroup
G = NSG * WPS  # 100 windows incl. dummies (98 real)
KD = 16  # dense slots per node
D = 32

_prog_cache = {}


def _split_hi_lo(x):
    hi = x.astype(BF16)
    lo = (x - hi.astype(np.float32)).astype(BF16)
    return hi, lo


def _host_prep(node_attr, edge_index, edge_attr, global_attr, W1, b1, W2, b2):
    E = edge_attr.shape[0]
    r = np.ascontiguousarray(edge_index[1]).astype(np.int64)

    order = np.argsort(r, kind="stable")
    r_s = r[order]
    deg = np.bincount(r, minlength=N_NODES)
    starts = np.zeros(N_NODES, dtype=np.int64)
    np.cumsum(deg[:-1], out=starts[1:])
    k = np.arange(E, dtype=np.int64) - starts[r_s]  # rank within node

    ea = np.ascontiguousarray(edge_attr, dtype=np.float32)
    hi, lo = _split_hi_lo(ea)
    hilo = np.concatenate([hi, lo], axis=1)[order]  # (E, 64) in sorted order

    core = r_s // NPC
    local = r_s - core * NPC
    sg = local // SG
    j = (local % SG) // P
    p = local % P
    w_in = (local % P).astype(np.float32)

    dense = k < KD
    arr_A = np.zeros((N_CORES, NSG, P, KD, WPS, 2 * D), dtype=BF16)
    arr_A[core[dense], sg[dense], p[dense], k[dense], j[dense]] = hilo[dense]

    # overflow: bucket per (core, window); windows 98,99 are dummies
    ov = ~dense
    wkey = (core * G + local // P).astype(np.int64)[ov]
    cnt = np.bincount(wkey, minlength=N_CORES * G)
    m_l = max(1, int(-(-cnt.max() // P)))
    starts2 = np.zeros(N_CORES * G, dtype=np.int64)
    np.cumsum(cnt[:-1], out=starts2[1:])
    # ov edges are already grouped by node hence by window in sorted order
    seq = np.zeros(E, dtype=np.int64)
    seq[ov] = np.arange(int(ov.sum()), dtype=np.int64)
    slot2 = seq[ov] - starts2[wkey]
    t2 = slot2 // P
    p2 = slot2 % P

    arr_B = np.zeros((N_CORES, NSG, P, WPS, m_l, 2 * D), dtype=BF16)
    arr_B[core[ov], sg[ov], p2, j[ov], t2] = hilo[ov]

    widx = np.full((N_CORES * G, m_l * P), -1.0, dtype=np.float32)
    widx[wkey, slot2] = w_in[ov]
    # (c, w=sg*4+j, t, p) -> (c, p, sg, j, t)
    idx_arr = np.ascontiguousarray(
        widx.reshape(N_CORES, NSG, WPS, m_l, P).transpose(0, 4, 1, 2, 3)
    ).reshape(N_CORES, P, NSG * WPS * m_l).astype(BF16)

    edges_in = np.concatenate(
        [
            arr_A.reshape(N_CORES, NSG, P, KD * WPS * 2 * D),
            arr_B.reshape(N_CORES, NSG, P, WPS * m_l * 2 * D),
        ],
        axis=3,
    )

    nodeT = np.ascontiguousarray(
        node_attr.astype(np.float32).reshape(N_CORES, NPC, D).transpose(0, 2, 1)
    )
    nhi, nlo = _split_hi_lo(nodeT)
    node_stack = np.concatenate([nhi, nhi, nlo], axis=1)  # (8, 96, NPC)

    g0 = global_attr.astype(np.float32).reshape(1, D)
    W1 = W1.astype(np.float32)
    b1p = (b1.astype(np.float32) + (g0 @ W1[2 * D :]).reshape(-1)).reshape(D, 1)
    w1a_hi, w1a_lo = _split_hi_lo(W1[:D])
    w1b_hi, w1b_lo = _split_hi_lo(W1[D : 2 * D])
    w1n = np.ascontiguousarray(np.concatenate([w1a_hi, w1a_lo, w1a_hi], axis=0))
    w1g = np.ascontiguousarray(np.concatenate([w1b_hi, w1b_lo, w1b_hi], axis=0))
    W2 = np.ascontiguousarray(W2.astype(np.float32))
    b2 = b2.astype(np.float32).reshape(D, 1)

    in_maps = []
    for c in range(N_CORES):
        in_maps.append(
            {
                "edges": edges_in[c],
                "idx": idx_arr[c],
                "nodeS": node_stack[c],
                "w1n": w1n,
                "w1g": w1g,
                "w2": W2,
                "b1p": b1p,
                "b2": b2,
            }
        )
    return in_maps, m_l


def _build_program(m_l):
    if m_l in _prog_cache:
        return _prog_cache[m_l]

    f32 = mybir.dt.float32
    bf16 = mybir.dt.bfloat16
    nc = bacc.Bacc(
        "TRN2", target_bir_lowering=False, debug=False, num_devices=N_CORES
    )

    A_ELEMS = KD * WPS * 2 * D  # 4096
    B_ELEMS = WPS * m_l * 2 * D
    NT = WPS * m_l  # overflow tiles per supergroup

    edges_d = nc.dram_tensor(
        "edges", [NSG, P, A_ELEMS + B_ELEMS], bf16, kind="ExternalInput"
    )
    idx_d = nc.dram_tensor("idx", [P, NSG * NT], bf16, kind="ExternalInput")
    nodeS_d = nc.dram_tensor("nodeS", [3 * D, NPC], bf16, kind="ExternalInput")
    w1n_d = nc.dram_tensor("w1n", [3 * D, D], bf16, kind="ExternalInput")
    w1g_d = nc.dram_tensor("w1g", [3 * D, D], bf16, kind="ExternalInput")
    w2_d = nc.dram_tensor("w2", [D, D], f32, kind="ExternalInput")
    b1p_d = nc.dram_tensor("b1p", [D, 1], f32, kind="ExternalInput")
    b2_d = nc.dram_tensor("b2", [D, 1], f32, kind="ExternalInput")
    outT_d = nc.dram_tensor("outT", [D, NPC], f32, kind="ExternalOutput")

    with tile.TileContext(nc) as tc:
        with (
            tc.tile_pool(name="const", bufs=1) as cpool,
            tc.tile_pool(name="edges", bufs=3) as epool,
            tc.tile_pool(name="oh", bufs=3) as opool,
            tc.tile_pool(name="mlp", bufs=2) as mpool,
            tc.tile_pool(name="psum", bufs=2, space="PSUM") as pspool,
        ):
            # constants
            iota32 = cpool.tile([P, NT, P], mybir.dt.int32)
            nc.gpsimd.iota(
                iota32[:], pattern=[[0, NT], [1, P]], base=0, channel_multiplier=0
            )
            iotab = cpool.tile([P, NT, P], bf16)
            nc.vector.tensor_copy(out=iotab[:], in_=iota32[:])
            identity = cpool.tile([P, P], f32)
            make_identity(nc, identity[:])
            identity_bf = cpool.tile([P, P], bf16)
            nc.vector.tensor_copy(out=identity_bf[:], in_=identity[:])

            idx_all = cpool.tile([P, NSG * NT], bf16)
            nc.sync.dma_start(out=idx_all[:], in_=idx_d.ap())
            nodeS_sb = cpool.tile([3 * D, NPC], bf16)
            nc.sync.dma_start(out=nodeS_sb[:], in_=nodeS_d.ap())
            w1n_sb = cpool.tile([3 * D, D], bf16)
            nc.sync.dma_start(out=w1n_sb[:], in_=w1n_d.ap())
            w1g_sb = cpool.tile([3 * D, D], bf16)
            nc.sync.dma_start(out=w1g_sb[:], in_=w1g_d.ap())
            w2_sb = cpool.tile([D, D], f32)
            nc.sync.dma_start(out=w2_sb[:], in_=w2_d.ap())
            b1p_sb = cpool.tile([D, 1], f32)
            nc.sync.dma_start(out=b1p_sb[:], in_=b1p_d.ap())
            b2_sb = cpool.tile([D, 1], f32)
            nc.sync.dma_start(out=b2_sb[:], in_=b2_d.ap())

            # agg stack [hi; hi; lo] built via DMA duplication at the end
            aggS = cpool.tile([3 * D, G * P], bf16)  # [96, 12800]
            aggL = cpool.tile([D, G * P], bf16)  # lo scratch [32, 12800]

            for s in range(NSG):
                edges_t = epool.tile([P, A_ELEMS + B_ELEMS], bf16)
                nc.sync.dma_start(out=edges_t[:], in_=edges_d.ap()[s])
                oh = opool.tile([P, NT, P], bf16)
                nc.vector.tensor_tensor(
                    out=oh[:],
                    in0=idx_all[:, s * NT : (s + 1) * NT].to_broadcast([P, NT, P]),
                    in1=iotab[:],
                    op=mybir.AluOpType.is_equal,
                )
                ps = pspool.tile([P, WPS, 2 * D], f32)
                for sl in range(KD):
                    nc.tensor.matmul(
                        out=ps[:],
                        lhsT=identity_bf[:],
                        rhs=edges_t[:, sl * 256 : (sl + 1) * 256],
                        start=(sl == 0),
                        stop=False,
                        skip_group_check=True,
                    )
                for jt in range(NT):
                    jj = jt // m_l
                    base = A_ELEMS + jt * 2 * D
                    nc.tensor.matmul(
                        out=ps[:, jj, :],
                        lhsT=oh[:, jt, :],
                        rhs=edges_t[:, base : base + 2 * D],
                        start=False,
                        stop=(jt == NT - 1),
                        skip_group_check=True,
                    )
                ps2 = opool.tile([P, WPS, 2 * D], f32, tag="ps2")
                nc.scalar.activation(
                    out=ps2[:], in_=ps[:], func=mybir.ActivationFunctionType.Copy
                )
                agg4 = opool.tile([P, WPS, D], f32, tag="agg4")
                nc.vector.tensor_tensor(
                    out=agg4[:],
                    in0=ps2[:, :, :D],
                    in1=ps2[:, :, D:],
                    op=mybir.AluOpType.add,
                )
                for jj in range(WPS):
                    pt = pspool.tile([D, P], f32, tag="pt")
                    nc.tensor.transpose(
                        out=pt[:], in_=agg4[:, jj, :], identity=identity[:]
                    )
                    w = s * WPS + jj
                    cols = slice(w * P, (w + 1) * P)
                    nc.scalar.activation(
                        out=aggS[:D, cols],
                        in_=pt[:],
                        func=mybir.ActivationFunctionType.Copy,
                    )
                    nc.vector.tensor_tensor(
                        out=aggL[:, cols],
                        in0=pt[:],
                        in1=aggS[:D, cols],
                        op=mybir.AluOpType.subtract,
                    )

            # duplicate hi to rows 32:64, lo to rows 64:96 (DMA moves partitions)
            nc.sync.dma_start(out=aggS[D : 2 * D, :], in_=aggS[:D, :])
            nc.sync.dma_start(out=aggS[2 * D :, :], in_=aggL[:])

            # MLP over node chunks
            CH = 512
            n_done = 0
            while n_done < NPC:
                n = min(CH, NPC - n_done)
                cols = slice(n_done, n_done + n)
                ph = pspool.tile([D, CH], f32, tag="ph")
                nc.tensor.matmul(
                    out=ph[:, :n],
                    lhsT=w1n_sb[:],
                    rhs=nodeS_sb[:, cols],
                    start=True,
                    stop=False,
                )
                nc.tensor.matmul(
                    out=ph[:, :n],
                    lhsT=w1g_sb[:],
                    rhs=aggS[:, cols],
                    start=False,
                    stop=True,
                )
                hT = mpool.tile([D, CH], f32, tag="hT")
                nc.scalar.activation(
                    out=hT[:, :n],
                    in_=ph[:, :n],
                    func=mybir.ActivationFunctionType.Relu,
                    bias=b1p_sb[:],
                    scale=1.0,
                )
                po = pspool.tile([D, CH], f32, tag="po")
                nc.tensor.matmul(
                    out=po[:, :n],
                    lhsT=w2_sb[:],
                    rhs=hT[:, :n],
                    start=True,
                    stop=True,
                )
                ot = mpool.tile([D, CH], f32, tag="ot")
                nc.vector.tensor_tensor(
                    out=ot[:, :n],
                    in0=po[:, :n],
                    in1=b2_sb[:].to_broadcast([D, n]),
                    op=mybir.AluOpType.add,
                )
                nc.sync.dma_start(out=outT_d.ap()[:, cols], in_=ot[:, :n])
                n_done += n

    nc.finalize()
    _prog_cache[m_l] = nc
    return nc


def kernel(**inputs):
    in_maps, m_l = _host_prep(**inputs)
    nc = _build_program(m_l)
    trace = bool(os.environ.get("KERNEL_TRACE"))
    res = run_bass_kernel_spmd(nc, in_maps, list(range(N_CORES)), trace=trace)
    if trace:
        print(f"HW exec time: {res.exec_time_ns} ns")
        print(f"mean exec time: {res.mean_exec_time_ns} ns")
    out = np.empty((N_NODES, D), dtype=np.float32)
    for c in range(N_CORES):
        out[c * NPC : (c + 1) * NPC] = res.results[c]["outT"].T
    return out
